# revision 13
# baseline (speedup 1.0000x reference)
"""DeepseekV4Compressor Trainium2 kernel.

Strategy (8 NeuronCores, SPMD):
  - Shard tokens: core i handles batch b=i//2, sequence half (i%2)*2048.
  - Host pre-transposes x shard -> xT [H, 2048] and W=concat(W_kv,W_gate)
    -> WT [H, 2048] so the contraction dim (H) lands on SBUF partitions
    with line-rate DMA (no on-chip transposes of the big operands).
  - On chip, kv/gate projections run as fp32r matmuls (full PE rate at
    free dim 512) producing [features, tokens] tiles; sigmoid gating,
    learned pooling (softmax(ape) weights precomputed on host), RMSNorm
    and partial rotary are fused behind the matmul stream.
  - Chunk-boundary pooling overlap (chunk c uses chunk c-1's "ov" half)
    is handled inside a core by a shifted add; the 8 per-core first
    chunks are patched up on the host from tiny aux outputs (pre-norm
    pooled vector, weighted ov carry, raw first-chunk gated projection).
"""

import numpy as np
from contextlib import ExitStack

B, S, H = 4, 4096, 4096
RATIO = 4
HD = 512
ROPE = 64
EPS = 1e-6
C = S // RATIO
NCORES = 8
O = 4 * HD  # 2048 projected features: kv(main|ov) 1024 + gate(main|ov) 1024


def _build(h=H, tok=2048, n_ctx=None):
    """Emit the per-core Bass program. h/tok shrinkable for simulation."""
    import concourse.bass as bass  # noqa: F401
    import concourse.tile as tile
    from concourse import bacc, mybir
    from concourse.masks import make_identity

    f32 = mybir.dt.float32
    f32r = mybir.dt.float32r
    AF = mybir.ActivationFunctionType
    ALU = mybir.AluOpType
    AX = mybir.AxisListType

    nh = h // 128            # h-tiles
    tmac = tok // 2          # tokens per macro block
    nb = max(1, tmac // 512)  # moving-dim blocks per macro
    tb_w = tmac // nb        # moving-dim width (512 full size)
    cc = tok // RATIO        # chunks per core
    cpm = cc // 2            # chunks per macro
    nct = (cc + 127) // 128  # chunk tiles for the end phase
    wchunk = 8               # h-tiles per W dma chunk
    nwc = nh // wchunk if nh >= wchunk else 1
    wchunk = min(wchunk, nh)

    nc = bacc.Bacc("TRN2", target_bir_lowering=False, debug=False)

    xt_d = nc.dram_tensor("xt", [h, tok], f32r, kind="ExternalInput")
    wt_d = nc.dram_tensor("wt", [h, O], f32r, kind="ExternalInput")
    w8t_d = nc.dram_tensor("w8t", [HD, 8], f32, kind="ExternalInput")
    nrm_d = nc.dram_tensor("nrm", [1, HD], f32, kind="ExternalInput")
    cos_d = nc.dram_tensor("cosr", [cc, ROPE // 2], f32, kind="ExternalInput")
    sin_d = nc.dram_tensor("sinr", [cc, ROPE // 2], f32, kind="ExternalInput")

    out_d = nc.dram_tensor("out", [cc, HD], f32, kind="ExternalOutput")
    p0_d = nc.dram_tensor("p0", [4, 128], f32, kind="ExternalOutput")
    v_d = nc.dram_tensor("vcarry", [4, 128], f32, kind="ExternalOutput")
    m0_d = nc.dram_tensor("m0", [4, 128, RATIO], f32, kind="ExternalOutput")

    with tile.TileContext(nc) as tc, ExitStack() as ctx:
        const = ctx.enter_context(tc.tile_pool(name="const", bufs=1))
        xpool = ctx.enter_context(tc.tile_pool(name="xpool", bufs=1))
        wpool = ctx.enter_context(tc.tile_pool(name="wpool", bufs=3))
        kvgp = ctx.enter_context(tc.tile_pool(name="kvgp", bufs=4))
        gsp = ctx.enter_context(tc.tile_pool(name="gsp", bufs=3))
        poolp = ctx.enter_context(tc.tile_pool(name="poolp", bufs=1))
        sovp = ctx.enter_context(tc.tile_pool(name="sovp", bufs=2))
        endp = ctx.enter_context(tc.tile_pool(name="endp", bufs=2))
        smallp = ctx.enter_context(tc.tile_pool(name="smallp", bufs=2))
        psum = ctx.enter_context(tc.tile_pool(name="psum", bufs=6, space="PSUM"))
        psum_tr = ctx.enter_context(tc.tile_pool(name="psum_tr", bufs=2, space="PSUM"))

        # --- constants ---
        ident = const.tile([128, 128], f32)
        make_identity(nc, ident)
        w8sb = const.tile([128, 4, 8], f32)  # w8sb[p, dt, r] = w8[r, dt*128+p]
        nc.sync.dma_start(out=w8sb, in_=w8t_d.rearrange("(dt p) r -> p dt r", p=128))
        nrm_b = const.tile([128, HD], f32)  # norm_w broadcast over partitions
        nc.sync.dma_start(out=nrm_b, in_=nrm_d.broadcast_to((128, HD)))
        pct = min(128, cc)
        cos_sb = const.tile([pct, nct, ROPE // 2], f32)
        sin_sb = const.tile([pct, nct, ROPE // 2], f32)
        nc.sync.dma_start(out=cos_sb, in_=cos_d.rearrange("(ct p) r -> p ct r", p=pct))
        nc.sync.dma_start(out=sin_sb, in_=sin_d.rearrange("(ct p) r -> p ct r", p=pct))

        eps_t = const.tile([128, 1], f32)
        nc.vector.memset(eps_t, float(EPS))
        pooled = poolp.tile([128, 4, cc], f32)     # [d-part, d-tile, chunk]
        carry01 = const.tile([128, 4], f32)        # macro0 ov carry per d-tile
        vaux = const.tile([128, 4], f32)
        m0aux = const.tile([128, 4, RATIO], f32)

        xt_dram = xt_d.rearrange("(hh p) t -> p hh t", p=128)
        wt_dram = wt_d.rearrange("(hh p) o -> p hh o", p=128)

        for m in range(2):
            t0 = m * tmac
            xt = xpool.tile([128, nh, tmac], f32r, tag="xt")
            nc.sync.dma_start(out=xt, in_=xt_dram[:, :, t0:t0 + tmac])

            for dt in range(4):
                kvg = {}
                for role, jk, jg in (("main", dt, dt + 8), ("ov", dt + 4, dt + 12)):
                    pk, pg = [], []
                    for j, pss in ((jk, pk), (jg, pg)):
                        for tb in range(nb):
                            pss.append(psum.tile([128, tb_w], f32, tag="mm",
                                                 name=f"mm_{j}_{tb}"))
                        for wc in range(nwc):
                            hs = wc * wchunk
                            wt = wpool.tile([128, wchunk, 128], f32r, tag="wt")
                            nc.sync.dma_start(
                                out=wt,
                                in_=wt_dram[:, hs:hs + wchunk,
                                            j * 128:(j + 1) * 128],
                            )
                            for tb in range(nb):
                                for hi in range(wchunk):
                                    nc.tensor.matmul(
                                        pss[tb],
                                        wt[:, hi, :],
                                        xt[:, hs + hi,
                                           tb * tb_w:(tb + 1) * tb_w],
                                        start=(wc == 0 and hi == 0),
                                        stop=(wc == nwc - 1
                                              and hi == wchunk - 1),
                                    )
                    kvgt = kvgp.tile([128, tmac], f32, tag="kvg")
                    for tb in range(nb):
                        gs = gsp.tile([128, tb_w], f32, tag="gs")
                        nc.scalar.activation(gs, pg[tb], AF.Sigmoid)
                        nc.vector.tensor_tensor(
                            kvgt[:, tb * tb_w:(tb + 1) * tb_w],
                            pk[tb], gs, ALU.mult)
                    kvg[role] = kvgt

                # --- pooling for this d-tile over this macro ---
                km, ko = kvg["main"], kvg["ov"]
                pm = pooled[:, dt, m * cpm:(m + 1) * cpm]
                nc.vector.tensor_scalar_mul(pm, km[:, 0::4], w8sb[:, dt, 0:1])
                for r in range(1, RATIO):
                    nc.vector.scalar_tensor_tensor(
                        pm, km[:, r::4], w8sb[:, dt, r:r + 1], pm,
                        ALU.mult, ALU.add)
                sov = sovp.tile([128, cpm], f32, tag="sov")
                nc.vector.tensor_scalar_mul(sov, ko[:, 0::4],
                                            w8sb[:, dt, 4:5])
                for r in range(1, RATIO):
                    nc.vector.scalar_tensor_tensor(
                        sov, ko[:, r::4], w8sb[:, dt, 4 + r:5 + r], sov,
                        ALU.mult, ALU.add)
                # shifted add: chunk c gets ov of chunk c-1
                tgt = pooled[:, dt, m * cpm + 1:(m + 1) * cpm]
                nc.vector.tensor_tensor(tgt, tgt, sov[:, 0:cpm - 1], ALU.add)
                if m == 0:
                    nc.vector.tensor_copy(carry01[:, dt:dt + 1],
                                          sov[:, cpm - 1:cpm])
                    nc.vector.tensor_copy(m0aux[:, dt, :], km[:, 0:RATIO])
                else:
                    mid = pooled[:, dt, cpm:cpm + 1]
                    nc.vector.tensor_tensor(mid, mid, carry01[:, dt:dt + 1],
                                            ALU.add)
                    nc.vector.tensor_copy(vaux[:, dt:dt + 1],
                                          sov[:, cpm - 1:cpm])

        # --- aux outputs ---
        nc.sync.dma_start(out=p0_d.rearrange("a b -> b a"),
                          in_=pooled[:, :, 0])
        nc.sync.dma_start(out=v_d.rearrange("a b -> b a"), in_=vaux)
        nc.sync.dma_start(out=m0_d.rearrange("a b r -> b a r"), in_=m0aux)

        # --- end phase: transpose pooled, RMSNorm, rotary, store ---
        for ct in range(nct):
            pc = min(pct, cc - ct * pct)
            pn = endp.tile([128, HD], f32, tag="pn")
            for dt in range(4):
                pst = psum_tr.tile([128, 128], f32, tag="tr")
                nc.tensor.transpose(
                    pst[:pc, :],
                    pooled[:, dt, ct * pct:ct * pct + pc], ident)
                nc.scalar.activation(pn[:pc, dt * 128:(dt + 1) * 128],
                                     pst[:pc, :], AF.Copy)
            sq = endp.tile([128, HD], f32, tag="scratch")
            nc.vector.tensor_tensor(sq[:pc], pn[:pc], pn[:pc], ALU.mult)
            var = smallp.tile([128, 1], f32, tag="var")
            nc.vector.reduce_sum(var[:pc], sq[:pc], axis=AX.X)
            rstd = smallp.tile([128, 1], f32, tag="rstd")
            nc.scalar.activation(rstd[:pc], var[:pc], AF.Sqrt,
                                 bias=eps_t[:pc], scale=1.0 / HD)
            nc.vector.reciprocal(rstd[:pc], rstd[:pc])
            ot = endp.tile([128, HD], f32, tag="scratch")
            nc.vector.scalar_tensor_tensor(
                ot[:pc], pn[:pc], rstd[:pc, 0:1], nrm_b[:pc],
                ALU.mult, ALU.mult)
            # partial interleaved rotary on last ROPE dims
            x2e = ot[:pc, HD - ROPE::2]
            x2o = ot[:pc, HD - ROPE + 1::2]
            ra = smallp.tile([128, ROPE // 2], f32, tag="ra")
            rb = smallp.tile([128, ROPE // 2], f32, tag="rb")
            rc = smallp.tile([128, ROPE // 2], f32, tag="rc")
            rd = smallp.tile([128, ROPE // 2], f32, tag="rd")
            cs, sn = cos_sb[:pc, ct, :], sin_sb[:pc, ct, :]
            nc.vector.tensor_tensor(ra[:pc], x2e, cs, ALU.mult)
            nc.vector.tensor_tensor(rb[:pc], x2o, sn, ALU.mult)
            nc.vector.tensor_tensor(rc[:pc], x2e, sn, ALU.mult)
            nc.vector.tensor_tensor(rd[:pc], x2o, cs, ALU.mult)
            nc.vector.tensor_sub(x2e, ra[:pc], rb[:pc])
            nc.vector.tensor_add(x2o, rc[:pc], rd[:pc])
            nc.sync.dma_start(out=out_d[ct * pct:ct * pct + pc, :],
                              in_=ot[:pc])

    nc.compile()
    return nc


_RUNNER = None


def _make_runner():
    """Build + compile once; return a reusable SPMD executor."""
    import jax
    import numpy as _np
    from jax.sharding import Mesh, PartitionSpec
    from jax.experimental.shard_map import shard_map
    from concourse import bass2jax, mybir

    nc = _build()
    bass2jax.install_neuronx_cc_hook()

    part_name = (nc.partition_id_tensor.name
                 if nc.partition_id_tensor is not None else None)
    in_names, out_names, out_avals, zero_outs = [], [], [], []
    for alloc in nc.m.functions[0].allocations:
        if not isinstance(alloc, mybir.MemoryLocationSet):
            continue
        name = alloc.memorylocations[0].name
        if alloc.kind == "ExternalInput":
            if name != part_name:
                in_names.append(name)
        elif alloc.kind == "ExternalOutput":
            out_names.append(name)
            shape = tuple(alloc.tensor_shape)
            dtype = mybir.dt.np(alloc.dtype)
            out_avals.append(jax.core.ShapedArray(shape, dtype))
            zero_outs.append(_np.zeros(shape, dtype))
    n_params = len(in_names)
    n_outs = len(out_avals)
    all_names = in_names + out_names
    if part_name is not None:
        all_names = all_names + [part_name]

    def _body(*args):
        operands = list(args)
        if part_name is not None:
            operands.append(bass2jax.partition_id_tensor())
        outs = bass2jax._bass_exec_p.bind(
            *operands,
            out_avals=tuple(out_avals),
            in_names=tuple(all_names),
            out_names=tuple(out_names),
            lowering_input_output_aliases=(),
            sim_require_finite=True,
            sim_require_nnan=True,
            nc=nc,
        )
        return tuple(outs)

    try:
        devices = jax.devices("axon")[:NCORES]
    except RuntimeError:
        try:
            devices = jax.devices("neuron")[:NCORES]
        except RuntimeError:
            devices = jax.devices()[:NCORES]
    mesh = Mesh(_np.asarray(devices), ("core",))
    sharding = jax.sharding.NamedSharding(mesh, PartitionSpec("core"))
    sharded = jax.jit(
        shard_map(
            _body, mesh=mesh,
            in_specs=(PartitionSpec("core"),) * (n_params + n_outs),
            out_specs=(PartitionSpec("core"),) * n_outs,
            check_rep=False,
        ),
        keep_unused=True,
    )

    class Runner:
        def prepare(self, in_maps):
            concat_in = [
                _np.concatenate([_np.asarray(m[name]) for m in in_maps],
                                axis=0)
                for name in in_names
            ]
            concat_zero = [
                _np.concatenate([z] * NCORES, axis=0) for z in zero_outs
            ]
            return [jax.device_put(a, sharding)
                    for a in concat_in + concat_zero]

        def run_prepared(self, dev_args):
            return jax.block_until_ready(sharded(*dev_args))

        def gather(self, outs):
            results = []
            for c in range(NCORES):
                res = {}
                for i, name in enumerate(out_names):
                    arr = _np.asarray(outs[i])
                    per = arr.shape[0] // NCORES
                    res[name] = arr[c * per:(c + 1) * per]
                results.append(res)
            return results

        def run(self, in_maps):
            return self.gather(self.run_prepared(self.prepare(in_maps)))

    return Runner()


def _softmax0(a):
    e = np.exp(a - a.max(axis=0, keepdims=True))
    return e / e.sum(axis=0, keepdims=True)


def _host_in_maps(x, cos, sin, W_kv, W_gate, ape, norm_w):
    W = np.concatenate([np.asarray(W_kv, np.float32),
                        np.asarray(W_gate, np.float32)], axis=0)
    WT = np.ascontiguousarray(W.T)                      # [H, 2048]
    ape = np.asarray(ape, np.float32)
    ape_comb = np.concatenate([ape[:, :HD], ape[:, HD:]], axis=0)
    w8 = _softmax0(ape_comb)                            # [8, HD]
    w8t = np.ascontiguousarray(w8.T)                    # [HD, 8]
    nrm = np.asarray(norm_w, np.float32).reshape(1, HD)
    x = np.asarray(x, np.float32)
    cos = np.asarray(cos, np.float32)
    sin = np.asarray(sin, np.float32)
    in_maps = []
    for core in range(NCORES):
        b, half = divmod(core, 2)
        s0 = half * (S // 2)
        c0 = half * (C // 2)
        xt = np.ascontiguousarray(x[b, s0:s0 + S // 2, :].T)  # [H, 2048]
        in_maps.append({
            "xt": xt, "wt": WT, "w8t": w8t, "nrm": nrm,
            "cosr": np.ascontiguousarray(cos[b, c0:c0 + C // 2]),
            "sinr": np.ascontiguousarray(sin[b, c0:c0 + C // 2]),
        })
    return in_maps, w8


def _host_fixup(out, results, cos, sin, ape, norm_w, w8):
    ape = np.asarray(ape, np.float32)
    w0 = _softmax0(ape[:, :HD])                         # [RATIO, HD]
    norm_w = np.asarray(norm_w, np.float32)
    for core in range(NCORES):
        b, half = divmod(core, 2)
        c0 = half * (C // 2)
        if half == 0:
            m0 = results[core]["m0"].reshape(HD, RATIO)
            pooled0 = (m0 * w0.T).sum(axis=1)
        else:
            p0 = results[core]["p0"].reshape(HD)
            v = results[core - 1]["vcarry"].reshape(HD)
            pooled0 = p0 + v
        var = np.mean(pooled0 * pooled0)
        o = pooled0 / np.sqrt(var + EPS) * norm_w
        x1, x2 = o[:HD - ROPE], o[HD - ROPE:]
        x2e, x2o = x2[0::2], x2[1::2]
        cb = np.asarray(cos, np.float32)[b, c0]
        sb = np.asarray(sin, np.float32)[b, c0]
        re = x2e * cb - x2o * sb
        ro = x2e * sb + x2o * cb
        out[b, c0] = np.concatenate(
            [x1, np.stack([re, ro], axis=-1).reshape(ROPE)])
    return out


def kernel(x, cos, sin, W_kv, W_gate, ape, norm_w):
    global _RUNNER
    if _RUNNER is None:
        _RUNNER = _make_runner()
    in_maps, w8 = _host_in_maps(x, cos, sin, W_kv, W_gate, ape, norm_w)
    results = _RUNNER.run(in_maps)
    out = np.zeros((B, C, HD), np.float32)
    for core in range(NCORES):
        b, half = divmod(core, 2)
        c0 = half * (C // 2)
        out[b, c0:c0 + C // 2] = results[core]["out"]
    return _host_fixup(out, results, cos, sin, ape, norm_w, w8)


# revision 26
# speedup vs baseline: 129.6340x; 129.6340x over previous
"""DeepseekV4Compressor Trainium2 kernel.

Strategy (8 NeuronCores, SPMD):
  - Shard tokens: core i handles batch b=i//2, sequence half (i%2)*2048.
  - Host pre-transposes x shard -> xT [H, 2048] and W=concat(W_kv,W_gate)
    -> WT [H, 2048] so the contraction dim (H) lands on SBUF partitions
    with line-rate DMA (no on-chip transposes of the big operands).
  - On chip, kv/gate projections run as fp32r matmuls (full PE rate at
    free dim 512) producing [features, tokens] tiles; sigmoid gating,
    learned pooling (softmax(ape) weights precomputed on host), RMSNorm
    and partial rotary are fused behind the matmul stream.
  - Chunk-boundary pooling overlap (chunk c uses chunk c-1's "ov" half)
    is handled inside a core by a shifted add; the 8 per-core first
    chunks are patched up on the host from tiny aux outputs (pre-norm
    pooled vector, weighted ov carry, raw first-chunk gated projection).
"""

import numpy as np
from contextlib import ExitStack

B, S, H = 4, 4096, 4096
RATIO = 4
HD = 512
ROPE = 64
EPS = 1e-6
C = S // RATIO
NCORES = 8
O = 4 * HD  # 2048 projected features: kv(main|ov) 1024 + gate(main|ov) 1024


def _build(h=H, tok=2048, reps=1):
    """Emit the per-core Bass program. h/tok shrinkable for simulation;
    reps>1 unrolls the whole computation for hardware timing probes."""
    import concourse.tile as tile
    from concourse import bacc, mybir
    from concourse.masks import make_identity

    f32 = mybir.dt.float32
    f32r = mybir.dt.float32r
    AF = mybir.ActivationFunctionType
    ALU = mybir.AluOpType
    AX = mybir.AxisListType

    nh = h // 128            # h-tiles
    tmac = tok // 2          # tokens per macro block
    nb = max(1, tmac // 512)  # moving-dim blocks per macro
    tb_w = tmac // nb        # moving-dim width (512 full size)
    cc = tok // RATIO        # chunks per core
    cpm = cc // 2            # chunks per macro
    nct = (cc + 127) // 128  # chunk tiles for the end phase
    wchunk = min(8, nh)      # h-tiles per W dma chunk
    nwc = nh // wchunk
    xchunk = min(4, nh)      # h-tiles per xt dma chunk
    nxc = nh // xchunk
    pct = min(128, cc)

    nc = bacc.Bacc("TRN2", target_bir_lowering=False, debug=False)

    xt_d = nc.dram_tensor("xt", [h, tok], f32r, kind="ExternalInput")
    wt_d = nc.dram_tensor("wt", [h, O], f32r, kind="ExternalInput")
    w8t_d = nc.dram_tensor("w8t", [HD, 8], f32, kind="ExternalInput")
    nrm_d = nc.dram_tensor("nrm", [1, HD], f32, kind="ExternalInput")
    cos_d = nc.dram_tensor("cosr", [cc, ROPE // 2], f32, kind="ExternalInput")
    sin_d = nc.dram_tensor("sinr", [cc, ROPE // 2], f32, kind="ExternalInput")

    out_d = nc.dram_tensor("out", [cc, HD], f32, kind="ExternalOutput")
    p0_d = nc.dram_tensor("p0", [4, 128], f32, kind="ExternalOutput")
    v_d = nc.dram_tensor("vcarry", [4, 128], f32, kind="ExternalOutput")
    m0_d = nc.dram_tensor("m0", [4, 128, RATIO], f32, kind="ExternalOutput")

    with tile.TileContext(nc) as tc, ExitStack() as ctx:
        const = ctx.enter_context(tc.tile_pool(name="const", bufs=1))
        xpool = ctx.enter_context(tc.tile_pool(name="xpool", bufs=8))
        wpool = ctx.enter_context(tc.tile_pool(name="wpool", bufs=4))
        kvgp = ctx.enter_context(tc.tile_pool(name="kvgp", bufs=4))
        gsp = ctx.enter_context(tc.tile_pool(name="gsp", bufs=3))
        poolp = ctx.enter_context(tc.tile_pool(name="poolp", bufs=1))
        sovp = ctx.enter_context(tc.tile_pool(name="sovp", bufs=2))
        endp = ctx.enter_context(tc.tile_pool(name="endp", bufs=2))
        smallp = ctx.enter_context(tc.tile_pool(name="smallp", bufs=2))
        psum = ctx.enter_context(tc.tile_pool(name="psum", bufs=6,
                                              space="PSUM"))
        psum_tr = ctx.enter_context(tc.tile_pool(name="psum_tr", bufs=2,
                                                 space="PSUM"))

        # --- constants ---
        ident = const.tile([128, 128], f32)
        make_identity(nc, ident)
        w8sb = const.tile([128, 4, 8], f32)  # w8sb[p, dt, r] = w8[r, dt*128+p]
        nc.sync.dma_start(out=w8sb,
                          in_=w8t_d.rearrange("(dt p) r -> p dt r", p=128))
        nrm_b = const.tile([128, HD], f32)  # norm_w broadcast over partitions
        nc.sync.dma_start(out=nrm_b, in_=nrm_d.broadcast_to((128, HD)))
        cos_sb = const.tile([pct, nct, ROPE // 2], f32)
        sin_sb = const.tile([pct, nct, ROPE // 2], f32)
        nc.sync.dma_start(out=cos_sb,
                          in_=cos_d.rearrange("(ct p) r -> p ct r", p=pct))
        nc.sync.dma_start(out=sin_sb,
                          in_=sin_d.rearrange("(ct p) r -> p ct r", p=pct))
        eps_t = const.tile([128, 1], f32)
        nc.vector.memset(eps_t, float(EPS))
        pooled = poolp.tile([128, 4, cc], f32)     # [d-part, d-tile, chunk]
        carry01 = const.tile([128, 4], f32)        # macro0 ov carry per d-tile
        vaux = const.tile([128, 4], f32)
        m0aux = const.tile([128, 4, RATIO], f32)

        xt_dram = xt_d.rearrange("(hh p) t -> p hh t", p=128)
        wt_dram = wt_d.rearrange("(hh p) o -> p hh o", p=128)

        early_cts = [ct for ct in range(nct) if (ct + 1) * pct <= cpm]
        late_cts = [ct for ct in range(nct) if (ct + 1) * pct > cpm]

        def emit_end(ct):
            pc = min(pct, cc - ct * pct)
            pn = endp.tile([128, HD], f32, tag="pn", name=f"pn_{ct}")
            for dt2 in range(4):
                pst = psum_tr.tile([128, 128], f32, tag="tr",
                                   name=f"tr_{ct}_{dt2}")
                nc.tensor.transpose(
                    pst[:pc, :],
                    pooled[:, dt2, ct * pct:ct * pct + pc], ident)
                nc.scalar.activation(pn[:pc, dt2 * 128:(dt2 + 1) * 128],
                                     pst[:pc, :], AF.Copy)
            sq = endp.tile([128, HD], f32, tag="scratch", name=f"sq_{ct}")
            nc.vector.tensor_tensor(sq[:pc], pn[:pc], pn[:pc], ALU.mult)
            var = smallp.tile([128, 1], f32, tag="var", name=f"var_{ct}")
            nc.vector.reduce_sum(var[:pc], sq[:pc], axis=AX.X)
            rstd = smallp.tile([128, 1], f32, tag="rstd", name=f"rstd_{ct}")
            nc.scalar.activation(rstd[:pc], var[:pc], AF.Sqrt,
                                 bias=eps_t[:pc], scale=1.0 / HD)
            nc.vector.reciprocal(rstd[:pc], rstd[:pc])
            ot = endp.tile([128, HD], f32, tag="scratch", name=f"ot_{ct}")
            nc.vector.scalar_tensor_tensor(
                ot[:pc], pn[:pc], rstd[:pc, 0:1], nrm_b[:pc],
                ALU.mult, ALU.mult)
            # partial interleaved rotary on last ROPE dims
            x2e = ot[:pc, HD - ROPE::2]
            x2o = ot[:pc, HD - ROPE + 1::2]
            ra = smallp.tile([128, ROPE // 2], f32, tag="ra", name=f"ra{ct}")
            rb = smallp.tile([128, ROPE // 2], f32, tag="rb", name=f"rb{ct}")
            rc = smallp.tile([128, ROPE // 2], f32, tag="rc", name=f"rc{ct}")
            rd = smallp.tile([128, ROPE // 2], f32, tag="rd", name=f"rd{ct}")
            cs, sn = cos_sb[:pc, ct, :], sin_sb[:pc, ct, :]
            nc.vector.tensor_tensor(ra[:pc], x2e, cs, ALU.mult)
            nc.vector.tensor_tensor(rb[:pc], x2o, sn, ALU.mult)
            nc.vector.tensor_tensor(rc[:pc], x2e, sn, ALU.mult)
            nc.vector.tensor_tensor(rd[:pc], x2o, cs, ALU.mult)
            nc.vector.tensor_sub(x2e, ra[:pc], rb[:pc])
            nc.vector.tensor_add(x2o, rc[:pc], rd[:pc])
            nc.sync.dma_start(out=out_d[ct * pct:ct * pct + pc, :],
                              in_=ot[:pc])

        for _rep in range(reps):
            for m in range(2):
                t0 = m * tmac
                xt_chunks = []
                for xc in range(nxc):
                    xtc = xpool.tile([128, xchunk, tmac], f32r, tag="xt",
                                     name=f"xt_{xc}")
                    nc.sync.dma_start(
                        out=xtc,
                        in_=xt_dram[:, xc * xchunk:(xc + 1) * xchunk,
                                    t0:t0 + tmac])
                    xt_chunks.append(xtc)

                for dt in range(4):
                    kvg = {}
                    for role, jk, jg in (("main", dt, dt + 8),
                                         ("ov", dt + 4, dt + 12)):
                        pk, pg = [], []
                        for j, pss in ((jk, pk), (jg, pg)):
                            for tb in range(nb):
                                pss.append(
                                    psum.tile([128, tb_w], f32, tag="mm",
                                              name=f"mm_{j}_{tb}"))
                            for wc in range(nwc):
                                hs = wc * wchunk
                                wt = wpool.tile([128, wchunk, 128], f32r,
                                                tag="wt")
                                # scalar HWDGE ring: keeps the W stream off
                                # the sync ring that carries the xt bursts
                                nc.scalar.dma_start(
                                    out=wt,
                                    in_=wt_dram[:, hs:hs + wchunk,
                                                j * 128:(j + 1) * 128],
                                )
                                for hi in range(wchunk):
                                    hh = hs + hi
                                    xtc = xt_chunks[hh // xchunk]
                                    for tb in range(nb):
                                        nc.tensor.matmul(
                                            pss[tb],
                                            wt[:, hi, :],
                                            xtc[:, hh % xchunk,
                                                tb * tb_w:(tb + 1) * tb_w],
                                            start=(wc == 0 and hi == 0),
                                            stop=(wc == nwc - 1
                                                  and hi == wchunk - 1),
                                        )
                        kvgt = kvgp.tile([128, tmac], f32, tag="kvg")
                        for tb in range(nb):
                            gs = gsp.tile([128, tb_w], f32, tag="gs")
                            nc.scalar.activation(gs, pg[tb], AF.Sigmoid)
                            nc.vector.tensor_tensor(
                                kvgt[:, tb * tb_w:(tb + 1) * tb_w],
                                pk[tb], gs, ALU.mult)
                        kvg[role] = kvgt

                    # --- pooling for this d-tile over this macro ---
                    km, ko = kvg["main"], kvg["ov"]
                    pm = pooled[:, dt, m * cpm:(m + 1) * cpm]
                    nc.vector.tensor_scalar_mul(pm, km[:, 0::4],
                                                w8sb[:, dt, 0:1])
                    for r in range(1, RATIO):
                        nc.vector.scalar_tensor_tensor(
                            pm, km[:, r::4], w8sb[:, dt, r:r + 1], pm,
                            ALU.mult, ALU.add)
                    sov = sovp.tile([128, cpm], f32, tag="sov")
                    nc.vector.tensor_scalar_mul(sov, ko[:, 0::4],
                                                w8sb[:, dt, 4:5])
                    for r in range(1, RATIO):
                        nc.vector.scalar_tensor_tensor(
                            sov, ko[:, r::4], w8sb[:, dt, 4 + r:5 + r], sov,
                            ALU.mult, ALU.add)
                    # shifted add: chunk c gets ov of chunk c-1
                    tgt = pooled[:, dt, m * cpm + 1:(m + 1) * cpm]
                    nc.vector.tensor_tensor(tgt, tgt, sov[:, 0:cpm - 1],
                                            ALU.add)
                    if m == 0:
                        nc.vector.tensor_copy(carry01[:, dt:dt + 1],
                                              sov[:, cpm - 1:cpm])
                        nc.vector.tensor_copy(m0aux[:, dt, :],
                                              km[:, 0:RATIO])
                    else:
                        mid = pooled[:, dt, cpm:cpm + 1]
                        nc.vector.tensor_tensor(mid, mid,
                                                carry01[:, dt:dt + 1],
                                                ALU.add)
                        nc.vector.tensor_copy(vaux[:, dt:dt + 1],
                                              sov[:, cpm - 1:cpm])

                if m == 0:
                    for ct in early_cts:
                        emit_end(ct)

            # --- aux outputs ---
            nc.sync.dma_start(out=p0_d.rearrange("a b -> b a"),
                              in_=pooled[:, :, 0])
            nc.sync.dma_start(out=v_d.rearrange("a b -> b a"), in_=vaux)
            nc.sync.dma_start(out=m0_d.rearrange("a b r -> b a r"),
                              in_=m0aux)

            # --- end phase: transpose pooled, RMSNorm, rotary, store ---
            for ct in late_cts:
                emit_end(ct)

    nc.compile()
    return nc


_RUNNER = None


def _make_runner(nc=None):
    """Build + compile once; return a reusable SPMD executor."""
    import jax
    import numpy as _np
    from jax.sharding import Mesh, PartitionSpec
    from jax.experimental.shard_map import shard_map
    from concourse import bass2jax, mybir

    if nc is None:
        nc = _build()
    bass2jax.install_neuronx_cc_hook()

    part_name = (nc.partition_id_tensor.name
                 if nc.partition_id_tensor is not None else None)
    in_names, out_names, out_avals, zero_outs = [], [], [], []
    for alloc in nc.m.functions[0].allocations:
        if not isinstance(alloc, mybir.MemoryLocationSet):
            continue
        name = alloc.memorylocations[0].name
        if alloc.kind == "ExternalInput":
            if name != part_name:
                in_names.append(name)
        elif alloc.kind == "ExternalOutput":
            out_names.append(name)
            shape = tuple(alloc.tensor_shape)
            dtype = mybir.dt.np(alloc.dtype)
            out_avals.append(jax.core.ShapedArray(shape, dtype))
            zero_outs.append(_np.zeros(shape, dtype))
    n_params = len(in_names)
    n_outs = len(out_avals)
    all_names = in_names + out_names
    if part_name is not None:
        all_names = all_names + [part_name]

    def _body(*args):
        operands = list(args)
        if part_name is not None:
            operands.append(bass2jax.partition_id_tensor())
        outs = bass2jax._bass_exec_p.bind(
            *operands,
            out_avals=tuple(out_avals),
            in_names=tuple(all_names),
            out_names=tuple(out_names),
            lowering_input_output_aliases=(),
            sim_require_finite=True,
            sim_require_nnan=True,
            nc=nc,
        )
        return tuple(outs)

    try:
        devices = jax.devices("axon")[:NCORES]
    except RuntimeError:
        try:
            devices = jax.devices("neuron")[:NCORES]
        except RuntimeError:
            devices = jax.devices()[:NCORES]
    mesh = Mesh(_np.asarray(devices), ("core",))
    sharding = jax.sharding.NamedSharding(mesh, PartitionSpec("core"))
    sharded = jax.jit(
        shard_map(
            _body, mesh=mesh,
            in_specs=(PartitionSpec("core"),) * (n_params + n_outs),
            out_specs=(PartitionSpec("core"),) * n_outs,
            check_rep=False,
        ),
        keep_unused=True,
    )

    class Runner:
        def prepare(self, in_maps):
            concat_in = [
                _np.concatenate([_np.asarray(m[name]) for m in in_maps],
                                axis=0)
                for name in in_names
            ]
            concat_zero = [
                _np.concatenate([z] * NCORES, axis=0) for z in zero_outs
            ]
            return [jax.device_put(a, sharding)
                    for a in concat_in + concat_zero]

        def run_prepared(self, dev_args):
            return jax.block_until_ready(sharded(*dev_args))

        def gather(self, outs):
            results = []
            for c in range(NCORES):
                res = {}
                for i, name in enumerate(out_names):
                    arr = _np.asarray(outs[i])
                    per = arr.shape[0] // NCORES
                    res[name] = arr[c * per:(c + 1) * per]
                results.append(res)
            return results

        def run(self, in_maps):
            return self.gather(self.run_prepared(self.prepare(in_maps)))

    return Runner()


def _softmax0(a):
    e = np.exp(a - a.max(axis=0, keepdims=True))
    return e / e.sum(axis=0, keepdims=True)


def _host_in_maps(x, cos, sin, W_kv, W_gate, ape, norm_w):
    W = np.concatenate([np.asarray(W_kv, np.float32),
                        np.asarray(W_gate, np.float32)], axis=0)
    WT = np.ascontiguousarray(W.T)                      # [H, 2048]
    ape = np.asarray(ape, np.float32)
    ape_comb = np.concatenate([ape[:, :HD], ape[:, HD:]], axis=0)
    w8 = _softmax0(ape_comb)                            # [8, HD]
    w8t = np.ascontiguousarray(w8.T)                    # [HD, 8]
    nrm = np.asarray(norm_w, np.float32).reshape(1, HD)
    x = np.asarray(x, np.float32)
    cos = np.asarray(cos, np.float32)
    sin = np.asarray(sin, np.float32)
    in_maps = []
    for core in range(NCORES):
        b, half = divmod(core, 2)
        s0 = half * (S // 2)
        c0 = half * (C // 2)
        xt = np.ascontiguousarray(x[b, s0:s0 + S // 2, :].T)  # [H, 2048]
        in_maps.append({
            "xt": xt, "wt": WT, "w8t": w8t, "nrm": nrm,
            "cosr": np.ascontiguousarray(cos[b, c0:c0 + C // 2]),
            "sinr": np.ascontiguousarray(sin[b, c0:c0 + C // 2]),
        })
    return in_maps, w8


def _host_fixup(out, results, cos, sin, ape, norm_w, w8):
    ape = np.asarray(ape, np.float32)
    w0 = _softmax0(ape[:, :HD])                         # [RATIO, HD]
    norm_w = np.asarray(norm_w, np.float32)
    for core in range(NCORES):
        b, half = divmod(core, 2)
        c0 = half * (C // 2)
        if half == 0:
            m0 = results[core]["m0"].reshape(HD, RATIO)
            pooled0 = (m0 * w0.T).sum(axis=1)
        else:
            p0 = results[core]["p0"].reshape(HD)
            v = results[core - 1]["vcarry"].reshape(HD)
            pooled0 = p0 + v
        var = np.mean(pooled0 * pooled0)
        o = pooled0 / np.sqrt(var + EPS) * norm_w
        x1, x2 = o[:HD - ROPE], o[HD - ROPE:]
        x2e, x2o = x2[0::2], x2[1::2]
        cb = np.asarray(cos, np.float32)[b, c0]
        sb = np.asarray(sin, np.float32)[b, c0]
        re = x2e * cb - x2o * sb
        ro = x2e * sb + x2o * cb
        out[b, c0] = np.concatenate(
            [x1, np.stack([re, ro], axis=-1).reshape(ROPE)])
    return out


def kernel(x, cos, sin, W_kv, W_gate, ape, norm_w):
    global _RUNNER
    if _RUNNER is None:
        _RUNNER = _make_runner()
    in_maps, w8 = _host_in_maps(x, cos, sin, W_kv, W_gate, ape, norm_w)
    results = _RUNNER.run(in_maps)
    out = np.zeros((B, C, HD), np.float32)
    for core in range(NCORES):
        b, half = divmod(core, 2)
        c0 = half * (C // 2)
        out[b, c0:c0 + C // 2] = results[core]["out"]
    return _host_fixup(out, results, cos, sin, ape, norm_w, w8)
